# revision 7
# baseline (speedup 1.0000x reference)
"""TSP attention-model (Kool-style AM) greedy decoder — 8-core data-parallel kernel.

Strategy: pure data parallel over the batch (512 -> 8 x 64) per the sharding
hint. The sequential 20-step decode re-encodes remaining nodes each step; the
only cross-shard coupling is BatchNorm's training-mode batch statistics, which
are computed exactly via cross-core mean (psum / all-reduce of per-shard sums).

This file is self-contained: shapes/constants are hardcoded from the problem
spec (B=512, N=20, D=128, H=8, L=3, FF=512, HID=512).
"""

import os
import numpy as np

D = 128; H = 8; L = 3; FF = 512; HID = 512
B = 512; N = 20
KD = D // H
CLIP = 10.0
M_CORES = 8
BL = B // M_CORES  # per-core batch


def _forward_np(coords, Wi, bi, W_ph, enc_Wq, enc_Wk, enc_Wv, enc_Wo, enc_g1, enc_b1,
                enc_ffW1, enc_ffb1, enc_ffW2, enc_ffb2, enc_g2, enc_b2,
                W_node, W_fixed, W_step, W_out, Wc1, bc1, Wc2, bc2, jnp, jax):
    """Exact port of the reference decode loop (same op order, same dtypes)."""

    def _bn(x, g, b):
        m = jnp.mean(x, axis=(0, 1)); v = jnp.var(x, axis=(0, 1))
        return (x - m) * jax.lax.rsqrt(v + 1e-5) * g + b

    def _mha(x, Wq, Wk, Wv, Wo):
        q = jnp.einsum('bnd,hdk->hbnk', x, Wq)
        k = jnp.einsum('bnd,hdk->hbnk', x, Wk)
        v = jnp.einsum('bnd,hdk->hbnk', x, Wv)
        a = jax.nn.softmax(jnp.einsum('hbnk,hbmk->hbnm', q, k) / np.float32(np.sqrt(KD)), axis=-1)
        h = jnp.einsum('hbnm,hbmk->hbnk', a, v)
        return jnp.einsum('hbnk,hkd->bnd', h, Wo)

    def _encode(x):
        for l in range(L):
            x = _bn(x + _mha(x, enc_Wq[l], enc_Wk[l], enc_Wv[l], enc_Wo[l]),
                    enc_g1[l], enc_b1[l])
            f = jnp.maximum(x @ enc_ffW1[l] + enc_ffb1[l], 0.0) @ enc_ffW2[l] + enc_ffb2[l]
            x = _bn(x + f, enc_g2[l], enc_b2[l])
        return x

    Bs, Ns, _ = coords.shape
    bid = jnp.arange(Bs)
    x = coords @ Wi + bi
    crd = coords
    node_idx = jnp.broadcast_to(jnp.arange(Ns)[None, :], (Bs, Ns))
    step_ctx = jnp.broadcast_to(W_ph[None, :], (Bs, 2 * D))
    ctx_q = step_ctx @ W_step
    log_ps = []; irs = []; vals = []; tours = []
    first = prev = cur = None
    for i in range(Ns):
        n = Ns - i
        E = _encode(x)
        g_mean = jnp.mean(E, axis=1)
        q = g_mean @ W_fixed + ctx_q
        gK, gV, lK = jnp.split(E @ W_node, 3, axis=-1)
        qh = q.reshape(Bs, H, KD)
        a = jax.nn.softmax(jnp.einsum('bhk,bnhk->bhn', qh, gK.reshape(Bs, n, H, KD)) / np.float32(np.sqrt(KD)), axis=-1)
        glimpse = jnp.einsum('bhn,bnhk->bhk', a, gV.reshape(Bs, n, H, KD)).reshape(Bs, D) @ W_out
        logits = jnp.tanh(jnp.einsum('bd,bnd->bn', glimpse, lK) / np.float32(np.sqrt(D))) * CLIP
        log_p = jax.nn.log_softmax(logits, axis=-1)
        sel = jnp.argmax(log_p, axis=1)
        log_ps.append(log_p[bid, sel])
        cur = crd[bid, sel]
        if i == 0:
            first = cur
            irs.append(jnp.zeros((Bs,), coords.dtype))
        else:
            irs.append(-jnp.sqrt(jnp.sum((cur - prev) ** 2, axis=1)))
        h = jnp.maximum(jnp.concatenate([g_mean, step_ctx], axis=1) @ Wc1 + bc1, 0.0)
        vals.append((h @ Wc2 + bc2)[:, 0])
        tours.append(node_idx[bid, sel])
        if i < Ns - 1:
            sel_mask = (jnp.arange(n)[None, :] == sel[:, None]).astype(jnp.int32)
            keep = jnp.argsort(sel_mask, axis=1)[:, : n - 1]
            crd = jnp.take_along_axis(crd, keep[:, :, None], axis=1)
            x = jnp.take_along_axis(x, keep[:, :, None], axis=1)
            node_idx = jnp.take_along_axis(node_idx, keep, axis=1)
        prev = cur
    reward_final = -jnp.sqrt(jnp.sum((first - cur) ** 2, axis=1))
    tours = jnp.stack(tours, axis=1)
    d = jnp.take_along_axis(coords, tours[:, :, None], axis=1)
    cost = jnp.sum(jnp.linalg.norm(d[:, 1:] - d[:, :-1], axis=2), axis=1) + jnp.linalg.norm(d[:, 0] - d[:, -1], axis=1)
    return (jnp.stack(log_ps, 1), jnp.stack(irs, 1), jnp.stack(vals, 1), cost, reward_final, tours)


NEG = np.float32(-1e9)
_JIT_CACHE = {}
_MESH_CACHE = {}


def _forward_sharded(inputs, jnp, jax, mesh):
    """8-way data-parallel decode via shard_map on the NeuronCores.

    Batch sharded 512 -> 8 x 64, params replicated. Uses a masked fixed-shape
    reformulation that is mathematically exact vs the compacting reference:

    - instead of removing the selected node each step, an availability mask
      (1 = still open) masks attention scores (-1e9 additive), token means
      (sum*mask / n), and BatchNorm statistics (masked sums / (B*n); the count
      B*n is deterministic). Compaction preserves token order, so greedy
      argmax picks the same node.
    - selection/gather ops (argmax / take_along_axis / argsort), unsupported
      or slow on trn2, become one-hot arithmetic: eq-compare against iota,
      min-reduce for the index, one-hot weighted sums for the gathers.
    - BatchNorm batch stats are global-exact via jax.lax.psum of the masked
      per-shard sums (the only cross-core communication, 2*D floats per BN).
    """
    from jax.sharding import PartitionSpec as P
    from jax.experimental.shard_map import shard_map

    param_keys = ['Wi', 'bi', 'W_ph', 'enc_Wq', 'enc_Wk', 'enc_Wv', 'enc_Wo',
                  'enc_g1', 'enc_b1', 'enc_ffW1', 'enc_ffb1', 'enc_ffW2',
                  'enc_ffb2', 'enc_g2', 'enc_b2', 'W_node', 'W_fixed',
                  'W_step', 'W_out', 'Wc1', 'bc1', 'Wc2', 'bc2']

    def local_forward(coords, params):
        (Wi, bi, W_ph, enc_Wq, enc_Wk, enc_Wv, enc_Wo, enc_g1, enc_b1,
         enc_ffW1, enc_ffb1, enc_ffW2, enc_ffb2, enc_g2, enc_b2,
         W_node, W_fixed, W_step, W_out, Wc1, bc1, Wc2, bc2) = params

        Bs = coords.shape[0]  # local batch (64)
        f32 = jnp.float32

        iota = jnp.arange(N, dtype=f32)[None, :]              # (1, N)

        def bn_masked(y, mask3, count, g, b):
            # y: (Bs,N,D), mask3: (Bs,N,1); exact global stats over open tokens
            s1 = jax.lax.psum(jnp.sum(y * mask3, axis=(0, 1)), 'c')
            m = s1 / count                                     # (D,)
            d = (y - m) * mask3
            s2 = jax.lax.psum(jnp.sum(d * d, axis=(0, 1)), 'c')
            v = s2 / count
            return (y - m) * jax.lax.rsqrt(v + 1e-5) * g + b

        def mha_masked(x, amask, Wq, Wk, Wv, Wo):
            # amask: (Bs,1,1,N) additive (-1e9 on closed keys)
            q = jnp.einsum('bnd,hdk->bhnk', x, Wq)
            k = jnp.einsum('bnd,hdk->bhnk', x, Wk)
            v = jnp.einsum('bnd,hdk->bhnk', x, Wv)
            s = jnp.einsum('bhnk,bhmk->bhnm', q, k) * np.float32(1.0 / np.sqrt(KD))
            s = s + amask
            p = jnp.exp(s - jax.lax.stop_gradient(jnp.max(s, axis=-1, keepdims=True)))
            p = p / jnp.sum(p, axis=-1, keepdims=True)
            h = jnp.einsum('bhnm,bhmk->bhnk', p, v)
            return jnp.einsum('bhnk,hkd->bnd', h, Wo)

        def encode(x, mask3, amask, count):
            for l in range(L):
                x = bn_masked(x + mha_masked(x, amask, enc_Wq[l], enc_Wk[l],
                                             enc_Wv[l], enc_Wo[l]),
                              mask3, count, enc_g1[l], enc_b1[l])
                f = jnp.maximum(x @ enc_ffW1[l] + enc_ffb1[l], 0.0) @ enc_ffW2[l] + enc_ffb2[l]
                x = bn_masked(x + f, mask3, count, enc_g2[l], enc_b2[l])
            return x

        x0 = coords @ Wi + bi                                  # (Bs,N,D)
        crd_x = coords[:, :, 0]
        crd_y = coords[:, :, 1]
        step_ctx_row = W_ph                                    # (2D,)
        ctx_q = (W_ph @ W_step)[None, :]                       # (1,D)
        cb1 = bc1 + step_ctx_row @ Wc1[D:, :]                  # fold const ctx into bias

        mask = jnp.ones((Bs, N), f32)                          # 1 = open
        log_ps = []; irs = []; vals = []; tours_f = []
        first_x = first_y = prev_x = prev_y = cur_x = cur_y = None
        for i in range(N):
            n = N - i
            count = np.float32(B * n)                          # global open count
            mask3 = mask[:, :, None]
            amask = ((mask - 1.0) * NEG * -1.0)[:, None, None, :]  # 0 open, -1e9 closed
            E = encode(x0, mask3, amask, count)
            g_mean = jnp.sum(E * mask3, axis=1) / np.float32(n)   # (Bs,D)
            q = g_mean @ W_fixed + ctx_q
            G = E @ W_node                                     # (Bs,N,3D)
            gK = G[:, :, :D]; gV = G[:, :, D:2 * D]; lK = G[:, :, 2 * D:]
            qh = q.reshape(Bs, H, KD)
            s = jnp.einsum('bhk,bnhk->bhn', qh, gK.reshape(Bs, N, H, KD)) * np.float32(1.0 / np.sqrt(KD))
            s = s + (mask - 1.0)[:, None, :] * -NEG
            p = jnp.exp(s - jnp.max(s, axis=-1, keepdims=True))
            p = p / jnp.sum(p, axis=-1, keepdims=True)
            glimpse = jnp.einsum('bhn,bnhk->bhk', p, gV.reshape(Bs, N, H, KD)).reshape(Bs, D) @ W_out
            logits = jnp.tanh(jnp.einsum('bd,bnd->bn', glimpse, lK) * np.float32(1.0 / np.sqrt(D))) * CLIP
            logits = logits + (mask - 1.0) * -NEG              # closed -> -1e9
            mx = jnp.max(logits, axis=-1, keepdims=True)
            ex = jnp.exp(logits - mx)
            lse = jnp.log(jnp.sum(ex, axis=-1, keepdims=True)) + mx
            log_p = logits - lse                               # (Bs,N)
            # manual first-argmax: min index among maxima
            is_max = (logits == mx).astype(f32)
            sel_f = jnp.min((1.0 - is_max) * np.float32(N) + iota * is_max, axis=-1,
                            keepdims=True)                     # (Bs,1)
            one_hot = (iota == sel_f).astype(f32) * mask       # (Bs,N)
            log_ps.append(jnp.sum(log_p * one_hot, axis=-1))
            cur_x = jnp.sum(crd_x * one_hot, axis=-1)
            cur_y = jnp.sum(crd_y * one_hot, axis=-1)
            if i == 0:
                first_x, first_y = cur_x, cur_y
                irs.append(jnp.zeros((Bs,), f32))
            else:
                irs.append(-jnp.sqrt((cur_x - prev_x) ** 2 + (cur_y - prev_y) ** 2))
            h = jnp.maximum(g_mean @ Wc1[:D, :] + cb1, 0.0)
            vals.append((h @ Wc2 + bc2)[:, 0])
            tours_f.append(jnp.sum(iota * one_hot, axis=-1))
            mask = mask * (1.0 - one_hot)
            prev_x, prev_y = cur_x, cur_y
        reward_final = -jnp.sqrt((first_x - cur_x) ** 2 + (first_y - cur_y) ** 2)
        dists = jnp.stack([-v for v in irs[1:]], axis=1)       # (Bs,19)
        cost = jnp.sum(dists, axis=1) + (-reward_final)
        tours = jnp.stack(tours_f, axis=1).astype(jnp.int32)
        return (jnp.stack(log_ps, 1), jnp.stack(irs, 1), jnp.stack(vals, 1),
                cost, reward_final, tours)

    params = tuple(jnp.asarray(inputs[k]) for k in param_keys)
    coords = jnp.asarray(inputs['coords'])

    key = id(mesh)
    fn = _JIT_CACHE.get(key)
    if fn is None:
        fn = jax.jit(shard_map(
            local_forward, mesh=mesh,
            in_specs=(P('c'), tuple(P() for _ in params)),
            out_specs=(P('c'), P('c'), P('c'), P('c'), P('c'), P('c')),
            check_rep=False,
        ))
        _JIT_CACHE[key] = fn
    return fn(coords, params)


def kernel(**inputs):
    import jax
    import jax.numpy as jnp

    use_device = os.environ.get('AM_KERNEL_DEVICE', '1') != '0'
    if use_device:
        try:
            if os.environ.get('AM_KERNEL_CPU_MESH', '0') == '1':
                devs = jax.devices('cpu')
            else:
                devs = [d for d in jax.devices() if d.platform != 'cpu']
            if len(devs) >= M_CORES:
                from jax.sharding import Mesh
                mkey = tuple(str(d) for d in devs[:M_CORES])
                mesh = _MESH_CACHE.get(mkey)
                if mesh is None:
                    mesh = Mesh(np.asarray(devs[:M_CORES]), ('c',))
                    _MESH_CACHE[mkey] = mesh
                outs = _forward_sharded(inputs, jnp, jax, mesh)
                return tuple(np.asarray(o) for o in outs)
        except Exception:
            if os.environ.get('AM_KERNEL_RAISE', '0') == '1':
                raise
            pass  # fall back to exact CPU path

    # CPU fallback: exact eager replication of the reference
    cpu = jax.devices('cpu')[0]
    with jax.default_device(cpu):
        args = [jnp.asarray(inputs[k]) for k in (
            'coords', 'Wi', 'bi', 'W_ph', 'enc_Wq', 'enc_Wk', 'enc_Wv', 'enc_Wo',
            'enc_g1', 'enc_b1', 'enc_ffW1', 'enc_ffb1', 'enc_ffW2', 'enc_ffb2',
            'enc_g2', 'enc_b2', 'W_node', 'W_fixed', 'W_step', 'W_out',
            'Wc1', 'bc1', 'Wc2', 'bc2')]
        outs = _forward_np(*args, jnp=jnp, jax=jax)
        return tuple(np.asarray(o) for o in outs)


# revision 9
# speedup vs baseline: 1.0457x; 1.0457x over previous
"""TSP attention-model (Kool-style AM) greedy decoder — 8-core data-parallel kernel.

Strategy: pure data parallel over the batch (512 -> 8 x 64) per the sharding
hint. The sequential 20-step decode re-encodes remaining nodes each step; the
only cross-shard coupling is BatchNorm's training-mode batch statistics, which
are computed exactly via cross-core mean (psum / all-reduce of per-shard sums).

This file is self-contained: shapes/constants are hardcoded from the problem
spec (B=512, N=20, D=128, H=8, L=3, FF=512, HID=512).
"""

import os
import numpy as np

D = 128; H = 8; L = 3; FF = 512; HID = 512
B = 512; N = 20
KD = D // H
CLIP = 10.0
M_CORES = 8
BL = B // M_CORES  # per-core batch


def _forward_np(coords, Wi, bi, W_ph, enc_Wq, enc_Wk, enc_Wv, enc_Wo, enc_g1, enc_b1,
                enc_ffW1, enc_ffb1, enc_ffW2, enc_ffb2, enc_g2, enc_b2,
                W_node, W_fixed, W_step, W_out, Wc1, bc1, Wc2, bc2, jnp, jax):
    """Exact port of the reference decode loop (same op order, same dtypes)."""

    def _bn(x, g, b):
        m = jnp.mean(x, axis=(0, 1)); v = jnp.var(x, axis=(0, 1))
        return (x - m) * jax.lax.rsqrt(v + 1e-5) * g + b

    def _mha(x, Wq, Wk, Wv, Wo):
        q = jnp.einsum('bnd,hdk->hbnk', x, Wq)
        k = jnp.einsum('bnd,hdk->hbnk', x, Wk)
        v = jnp.einsum('bnd,hdk->hbnk', x, Wv)
        a = jax.nn.softmax(jnp.einsum('hbnk,hbmk->hbnm', q, k) / np.float32(np.sqrt(KD)), axis=-1)
        h = jnp.einsum('hbnm,hbmk->hbnk', a, v)
        return jnp.einsum('hbnk,hkd->bnd', h, Wo)

    def _encode(x):
        for l in range(L):
            x = _bn(x + _mha(x, enc_Wq[l], enc_Wk[l], enc_Wv[l], enc_Wo[l]),
                    enc_g1[l], enc_b1[l])
            f = jnp.maximum(x @ enc_ffW1[l] + enc_ffb1[l], 0.0) @ enc_ffW2[l] + enc_ffb2[l]
            x = _bn(x + f, enc_g2[l], enc_b2[l])
        return x

    Bs, Ns, _ = coords.shape
    bid = jnp.arange(Bs)
    x = coords @ Wi + bi
    crd = coords
    node_idx = jnp.broadcast_to(jnp.arange(Ns)[None, :], (Bs, Ns))
    step_ctx = jnp.broadcast_to(W_ph[None, :], (Bs, 2 * D))
    ctx_q = step_ctx @ W_step
    log_ps = []; irs = []; vals = []; tours = []
    first = prev = cur = None
    for i in range(Ns):
        n = Ns - i
        E = _encode(x)
        g_mean = jnp.mean(E, axis=1)
        q = g_mean @ W_fixed + ctx_q
        gK, gV, lK = jnp.split(E @ W_node, 3, axis=-1)
        qh = q.reshape(Bs, H, KD)
        a = jax.nn.softmax(jnp.einsum('bhk,bnhk->bhn', qh, gK.reshape(Bs, n, H, KD)) / np.float32(np.sqrt(KD)), axis=-1)
        glimpse = jnp.einsum('bhn,bnhk->bhk', a, gV.reshape(Bs, n, H, KD)).reshape(Bs, D) @ W_out
        logits = jnp.tanh(jnp.einsum('bd,bnd->bn', glimpse, lK) / np.float32(np.sqrt(D))) * CLIP
        log_p = jax.nn.log_softmax(logits, axis=-1)
        sel = jnp.argmax(log_p, axis=1)
        log_ps.append(log_p[bid, sel])
        cur = crd[bid, sel]
        if i == 0:
            first = cur
            irs.append(jnp.zeros((Bs,), coords.dtype))
        else:
            irs.append(-jnp.sqrt(jnp.sum((cur - prev) ** 2, axis=1)))
        h = jnp.maximum(jnp.concatenate([g_mean, step_ctx], axis=1) @ Wc1 + bc1, 0.0)
        vals.append((h @ Wc2 + bc2)[:, 0])
        tours.append(node_idx[bid, sel])
        if i < Ns - 1:
            sel_mask = (jnp.arange(n)[None, :] == sel[:, None]).astype(jnp.int32)
            keep = jnp.argsort(sel_mask, axis=1)[:, : n - 1]
            crd = jnp.take_along_axis(crd, keep[:, :, None], axis=1)
            x = jnp.take_along_axis(x, keep[:, :, None], axis=1)
            node_idx = jnp.take_along_axis(node_idx, keep, axis=1)
        prev = cur
    reward_final = -jnp.sqrt(jnp.sum((first - cur) ** 2, axis=1))
    tours = jnp.stack(tours, axis=1)
    d = jnp.take_along_axis(coords, tours[:, :, None], axis=1)
    cost = jnp.sum(jnp.linalg.norm(d[:, 1:] - d[:, :-1], axis=2), axis=1) + jnp.linalg.norm(d[:, 0] - d[:, -1], axis=1)
    return (jnp.stack(log_ps, 1), jnp.stack(irs, 1), jnp.stack(vals, 1), cost, reward_final, tours)


NEG = np.float32(-1e9)
_JIT_CACHE = {}
_MESH_CACHE = {}


def _forward_sharded(inputs, jnp, jax, mesh):
    """8-way data-parallel decode via shard_map on the NeuronCores.

    Batch sharded 512 -> 8 x 64, params replicated. Uses a masked fixed-shape
    reformulation that is mathematically exact vs the compacting reference:

    - instead of removing the selected node each step, an availability mask
      (1 = still open) masks attention scores (-1e9 additive), token means
      (sum*mask / n), and BatchNorm statistics (masked sums / (B*n); the count
      B*n is deterministic). Compaction preserves token order, so greedy
      argmax picks the same node.
    - selection/gather ops (argmax / take_along_axis / argsort), unsupported
      or slow on trn2, become one-hot arithmetic: eq-compare against iota,
      min-reduce for the index, one-hot weighted sums for the gathers.
    - BatchNorm batch stats are global-exact via jax.lax.psum of the masked
      per-shard sums (the only cross-core communication, 2*D floats per BN).
    """
    from jax.sharding import PartitionSpec as P
    from jax.experimental.shard_map import shard_map

    param_keys = ['Wi', 'bi', 'W_ph', 'enc_Wq', 'enc_Wk', 'enc_Wv', 'enc_Wo',
                  'enc_g1', 'enc_b1', 'enc_ffW1', 'enc_ffb1', 'enc_ffW2',
                  'enc_ffb2', 'enc_g2', 'enc_b2', 'W_node', 'W_fixed',
                  'W_step', 'W_out', 'Wc1', 'bc1', 'Wc2', 'bc2']

    def local_forward(coords, params):
        (Wi, bi, W_ph, enc_Wq, enc_Wk, enc_Wv, enc_Wo, enc_g1, enc_b1,
         enc_ffW1, enc_ffb1, enc_ffW2, enc_ffb2, enc_g2, enc_b2,
         W_node, W_fixed, W_step, W_out, Wc1, bc1, Wc2, bc2) = params

        Bs = coords.shape[0]  # local batch (64)
        f32 = jnp.float32

        iota = jnp.arange(N, dtype=f32)[None, :]              # (1, N)

        def bn_masked(y, mask3, count, g, b):
            # y: (Bs,N,D); one-pass masked stats, single fused all-reduce:
            # psum of [sum(y), sum(y^2)] over open tokens; var = E[y^2]-m^2
            ym = y * mask3
            loc = jnp.concatenate([jnp.sum(ym, axis=(0, 1)),
                                   jnp.sum(ym * y, axis=(0, 1))])
            tot = jax.lax.psum(loc, 'c')
            m = tot[:D] / count
            v = tot[D:] / count - m * m
            return (y - m) * jax.lax.rsqrt(v + 1e-5) * g + b

        def mha_masked(x, amask, Wqkv, Wo):
            # amask: (Bs,1,1,N) additive (-1e9 on closed keys)
            qkv = x @ Wqkv                                     # (Bs,N,3*D)
            q = qkv[:, :, :D].reshape(Bs, N, H, KD).transpose(0, 2, 1, 3)
            k = qkv[:, :, D:2 * D].reshape(Bs, N, H, KD).transpose(0, 2, 1, 3)
            v = qkv[:, :, 2 * D:].reshape(Bs, N, H, KD).transpose(0, 2, 1, 3)
            s = jnp.einsum('bhnk,bhmk->bhnm', q, k) * np.float32(1.0 / np.sqrt(KD))
            s = s + amask
            p = jnp.exp(s - jax.lax.stop_gradient(jnp.max(s, axis=-1, keepdims=True)))
            p = p / jnp.sum(p, axis=-1, keepdims=True)
            h = jnp.einsum('bhnm,bhmk->bhnk', p, v)
            return jnp.einsum('bhnk,hkd->bnd', h, Wo)

        # merged per-layer QKV weights: (D, 3D), columns (h,k)-ordered so the
        # head reshape matches einsum('bnd,hdk->bhnk'); contraction per output
        # column is unchanged, so results are bit-identical to separate matmuls
        Wqkv_all = [jnp.concatenate([W[l].transpose(1, 0, 2).reshape(D, D)
                                     for W in (enc_Wq, enc_Wk, enc_Wv)], axis=1)
                    for l in range(L)]

        def encode(x, mask3, amask, count):
            for l in range(L):
                x = bn_masked(x + mha_masked(x, amask, Wqkv_all[l], enc_Wo[l]),
                              mask3, count, enc_g1[l], enc_b1[l])
                f = jnp.maximum(x @ enc_ffW1[l] + enc_ffb1[l], 0.0) @ enc_ffW2[l] + enc_ffb2[l]
                x = bn_masked(x + f, mask3, count, enc_g2[l], enc_b2[l])
            return x

        x0 = coords @ Wi + bi                                  # (Bs,N,D)
        crd_x = coords[:, :, 0]
        crd_y = coords[:, :, 1]
        step_ctx_row = W_ph                                    # (2D,)
        ctx_q = (W_ph @ W_step)[None, :]                       # (1,D)
        cb1 = bc1 + step_ctx_row @ Wc1[D:, :]                  # fold const ctx into bias

        mask = jnp.ones((Bs, N), f32)                          # 1 = open
        log_ps = []; irs = []; vals = []; tours_f = []
        first_x = first_y = prev_x = prev_y = cur_x = cur_y = None
        for i in range(N):
            n = N - i
            count = np.float32(B * n)                          # global open count
            mask3 = mask[:, :, None]
            amask = ((mask - 1.0) * NEG * -1.0)[:, None, None, :]  # 0 open, -1e9 closed
            E = encode(x0, mask3, amask, count)
            g_mean = jnp.sum(E * mask3, axis=1) / np.float32(n)   # (Bs,D)
            q = g_mean @ W_fixed + ctx_q
            G = E @ W_node                                     # (Bs,N,3D)
            gK = G[:, :, :D]; gV = G[:, :, D:2 * D]; lK = G[:, :, 2 * D:]
            qh = q.reshape(Bs, H, KD)
            s = jnp.einsum('bhk,bnhk->bhn', qh, gK.reshape(Bs, N, H, KD)) * np.float32(1.0 / np.sqrt(KD))
            s = s + (mask - 1.0)[:, None, :] * -NEG
            p = jnp.exp(s - jnp.max(s, axis=-1, keepdims=True))
            p = p / jnp.sum(p, axis=-1, keepdims=True)
            glimpse = jnp.einsum('bhn,bnhk->bhk', p, gV.reshape(Bs, N, H, KD)).reshape(Bs, D) @ W_out
            logits = jnp.tanh(jnp.einsum('bd,bnd->bn', glimpse, lK) * np.float32(1.0 / np.sqrt(D))) * CLIP
            logits = logits + (mask - 1.0) * -NEG              # closed -> -1e9
            mx = jnp.max(logits, axis=-1, keepdims=True)
            ex = jnp.exp(logits - mx)
            lse = jnp.log(jnp.sum(ex, axis=-1, keepdims=True)) + mx
            log_p = logits - lse                               # (Bs,N)
            # manual first-argmax: min index among maxima
            is_max = (logits == mx).astype(f32)
            sel_f = jnp.min((1.0 - is_max) * np.float32(N) + iota * is_max, axis=-1,
                            keepdims=True)                     # (Bs,1)
            one_hot = (iota == sel_f).astype(f32) * mask       # (Bs,N)
            log_ps.append(jnp.sum(log_p * one_hot, axis=-1))
            cur_x = jnp.sum(crd_x * one_hot, axis=-1)
            cur_y = jnp.sum(crd_y * one_hot, axis=-1)
            if i == 0:
                first_x, first_y = cur_x, cur_y
                irs.append(jnp.zeros((Bs,), f32))
            else:
                irs.append(-jnp.sqrt((cur_x - prev_x) ** 2 + (cur_y - prev_y) ** 2))
            h = jnp.maximum(g_mean @ Wc1[:D, :] + cb1, 0.0)
            vals.append((h @ Wc2 + bc2)[:, 0])
            tours_f.append(jnp.sum(iota * one_hot, axis=-1))
            mask = mask * (1.0 - one_hot)
            prev_x, prev_y = cur_x, cur_y
        reward_final = -jnp.sqrt((first_x - cur_x) ** 2 + (first_y - cur_y) ** 2)
        dists = jnp.stack([-v for v in irs[1:]], axis=1)       # (Bs,19)
        cost = jnp.sum(dists, axis=1) + (-reward_final)
        tours = jnp.stack(tours_f, axis=1).astype(jnp.int32)
        return (jnp.stack(log_ps, 1), jnp.stack(irs, 1), jnp.stack(vals, 1),
                cost, reward_final, tours)

    params = tuple(jnp.asarray(inputs[k]) for k in param_keys)
    coords = jnp.asarray(inputs['coords'])

    key = id(mesh)
    fn = _JIT_CACHE.get(key)
    if fn is None:
        fn = jax.jit(shard_map(
            local_forward, mesh=mesh,
            in_specs=(P('c'), tuple(P() for _ in params)),
            out_specs=(P('c'), P('c'), P('c'), P('c'), P('c'), P('c')),
            check_rep=False,
        ))
        _JIT_CACHE[key] = fn
    return fn(coords, params)


def kernel(**inputs):
    import jax
    import jax.numpy as jnp

    use_device = os.environ.get('AM_KERNEL_DEVICE', '1') != '0'
    if use_device:
        try:
            if os.environ.get('AM_KERNEL_CPU_MESH', '0') == '1':
                devs = jax.devices('cpu')
            else:
                devs = [d for d in jax.devices() if d.platform != 'cpu']
            if len(devs) >= M_CORES:
                from jax.sharding import Mesh
                mkey = tuple(str(d) for d in devs[:M_CORES])
                mesh = _MESH_CACHE.get(mkey)
                if mesh is None:
                    mesh = Mesh(np.asarray(devs[:M_CORES]), ('c',))
                    _MESH_CACHE[mkey] = mesh
                outs = _forward_sharded(inputs, jnp, jax, mesh)
                return tuple(np.asarray(o) for o in outs)
        except Exception:
            if os.environ.get('AM_KERNEL_RAISE', '0') == '1':
                raise
            pass  # fall back to exact CPU path

    # CPU fallback: exact eager replication of the reference
    cpu = jax.devices('cpu')[0]
    with jax.default_device(cpu):
        args = [jnp.asarray(inputs[k]) for k in (
            'coords', 'Wi', 'bi', 'W_ph', 'enc_Wq', 'enc_Wk', 'enc_Wv', 'enc_Wo',
            'enc_g1', 'enc_b1', 'enc_ffW1', 'enc_ffb1', 'enc_ffW2', 'enc_ffb2',
            'enc_g2', 'enc_b2', 'W_node', 'W_fixed', 'W_step', 'W_out',
            'Wc1', 'bc1', 'Wc2', 'bc2')]
        outs = _forward_np(*args, jnp=jnp, jax=jax)
        return tuple(np.asarray(o) for o in outs)
